# revision 36
# baseline (speedup 1.0000x reference)
"""Trainium2 Bass kernel for AttentionalAggregation (segment softmax-weighted sum).

reference math:
    s = values @ gate_w + gate_b            # [N,1]
    w = segment_softmax(s, indices)         # [N,1]
    out = segment_sum(w * (values @ attn_w + attn_b))   # [G,EMB]

Algebraic restructuring (exact up to fp rounding):
  softmax weights per segment sum to 1, so
      out[g] = (U[g]/D[g]) @ attn_w + attn_b
  with U[g] = sum_{i in g} e_i * values_i, D[g] = sum_{i in g} e_i,
  e_i = exp(values_i . gate_w).  gate_b and the per-segment max shift
  cancel in the U/D ratio (|s| <= ~4 for this data, exp can't overflow).

Sharding: indices are sorted, so each of the 8 cores owns G/8 contiguous
segments and their (contiguous) nodes. No collectives.

v2 layout: each 128-node block carries 258 bf16 columns
  [ v (256) | 1.0 (1) | one-hot (16) ]
so ONE ~2.2MB DMA per 32-block group feeds everything (v for the PE
matmul rhs incl. the D ones-column, one-hot for the segment masks).

Per group the gate-dot s[p, j] = v . gate_w runs as:
  - DVE batched tensor_mul  prod = v * gate        (bf16 2x mode)
  - fold tree: prod 256 -> 128 -> 64 -> 32 by pairwise adds, with the
    first fold split DVE/GPSIMD by block ranges (tunable balance), the
    rest on the two engines by range
  - one batched DVE tensor_reduce [128, 32, 32] -> s (1x but only 32 wide)
This costs ~2x less DVE time than mult + full-width 1x reduce.

ACT: exp per group; GPSIMD: pe = onehot * e broadcast-multiply.
PE per block: LDWEIGHTS pe[:, j, :] (16 cols) + MATMUL uw[16, 257] over
rhs [v|1] accumulated across the window's blocks in PSUM; window epilogue
transposes U into per-core staging and copies the D column; every 8
windows one 128-segment output group runs (2 matmuls + 1/D scale + bias
+ DMA out), overlapping the tail with the stream.
"""

import numpy as np
import ml_dtypes

P = 128
EMB = 256
EMBA = EMB + 1      # v plus the ones column
COLS = EMBA + 1     # v + ones + pad (4B-aligned block strides); oh ships separately
HALF = 128
SEGW = 16           # segments per window == one-hot width
NCORES = 8
BLK = 16            # blocks per DMA group
GRP = 128           # segments per final-matmul group
MULT_DVE = 16       # blocks whose gate-dot multiply runs on DVE (rest GPSIMD)
RED_DVE = 16        # blocks whose free-dim reduce runs on DVE (rest ACT)
VEC_W = 128         # emb width of the vector gate-dot path; PE covers the rest
PE_W = EMB - VEC_W

_CACHE = {}


# ----------------------------------------------------------------------------
# Host-side preparation: shard + pad nodes into (core, window, block) layout.
# ----------------------------------------------------------------------------
def prepare_host(values, indices, G):
    idx = np.ascontiguousarray(np.asarray(indices).astype(np.int64))
    counts = np.bincount(idx, minlength=G)
    seg_start = np.zeros(G + 1, dtype=np.int64)
    np.cumsum(counts, out=seg_start[1:])

    assert G % NCORES == 0
    spc = G // NCORES                      # segments per core
    win_lo = list(range(0, spc, SEGW))
    win_w = [min(SEGW, spc - lo) for lo in win_lo]
    W = len(win_lo)

    b_w = []
    for w in range(W):
        need = 1
        for c in range(NCORES):
            s0 = c * spc + win_lo[w]
            n = int(seg_start[s0 + win_w[w]] - seg_start[s0])
            need = max(need, (n + P - 1) // P)
        b_w.append(need)
    nblk = sum(b_w)

    vals = np.asarray(values, dtype=np.float32)
    n_dma = (nblk + BLK - 1) // BLK
    nblk_pad = n_dma * BLK
    per_core = []
    for c in range(NCORES):
        v_pad = np.zeros((nblk_pad * P, COLS), dtype=ml_dtypes.bfloat16)
        oh = np.zeros((P, nblk_pad, SEGW), dtype=ml_dtypes.bfloat16)
        gb = 0
        for w in range(W):
            s0 = c * spc + win_lo[w]
            lo = int(seg_start[s0])
            hi = int(seg_start[s0 + win_w[w]])
            r = lo
            for b in range(b_w[w]):
                n = min(P, hi - r)
                if n > 0:
                    rows = slice(gb * P, gb * P + n)
                    v_pad[rows, 0:EMB] = vals[r : r + n]
                    v_pad[rows, EMB] = 1.0
                    loc = (idx[r : r + n] - s0).astype(np.int64)
                    oh[np.arange(n), gb, loc] = 1.0
                r += n
                gb += 1
        # regroup so each DMA group's data is per-partition-contiguous:
        # [g, n, p, d] -> [g, p, n, d]
        v_pad = np.ascontiguousarray(
            v_pad.reshape(n_dma, BLK, P, COLS).transpose(0, 2, 1, 3)
        ).reshape(n_dma * P, BLK * COLS)
        oh = np.ascontiguousarray(oh).reshape(P, nblk_pad * SEGW)
        # v^T for emb[128:256]: partition = emb dim; within group g columns
        # run (j * P + p) so block j's 128 columns form the LDWEIGHTS tile
        # whose gate2 matvec yields s2 for block j partition-parallel.
        v5 = v_pad.reshape(n_dma, P, BLK, COLS)
        vth = np.ascontiguousarray(
            v5[:, :, :, VEC_W:EMB].transpose(3, 0, 2, 1)
        ).reshape(PE_W, n_dma * BLK * P)
        per_core.append({"v": v_pad, "oh": oh, "vth": vth})
    meta = {"W": W, "b_w": tuple(b_w), "win_lo": tuple(win_lo),
            "win_w": tuple(win_w), "nblk": nblk, "spc": spc, "n_dma": n_dma}
    return per_core, meta


# ----------------------------------------------------------------------------
# Bass program (identical for all cores; data differs per core).
# ----------------------------------------------------------------------------
def build_bass(meta, reps=1):
    import concourse.bass as bass
    import concourse.bacc as bacc
    import concourse.tile as tile
    from concourse import mybir
    from concourse.bass import broadcast_tensor_aps
    from contextlib import ExitStack

    f32 = mybir.dt.float32
    bf16 = mybir.dt.bfloat16
    fp16 = mybir.dt.float16
    Act = mybir.ActivationFunctionType
    Alu = mybir.AluOpType

    W = meta["W"]
    b_w = meta["b_w"]
    win_lo = meta["win_lo"]
    win_w = meta["win_w"]
    nblk = meta["nblk"]
    spc = meta["spc"]
    n_dma = meta["n_dma"]
    n_grp = (spc + GRP - 1) // GRP
    win_per_grp = GRP // SEGW
    assert spc % GRP == 0 and W * SEGW == spc

    nc = bacc.Bacc(
        "TRN2",
        target_bir_lowering=False,
        debug=False,
        enable_asserts=False,
        num_devices=NCORES,
    )

    v_d = nc.dram_tensor("v", [n_dma * P, BLK * COLS], bf16,
                         kind="ExternalInput").ap()
    oh_d = nc.dram_tensor("oh", [P, n_dma * BLK * SEGW], bf16,
                          kind="ExternalInput").ap()
    vth_d = nc.dram_tensor("vth", [PE_W, n_dma * P * BLK], bf16,
                           kind="ExternalInput").ap()
    gate2_d = nc.dram_tensor("gate2", [PE_W, 1], bf16, kind="ExternalInput").ap()
    gate_d = nc.dram_tensor("gate_rep", [P, EMB], bf16, kind="ExternalInput").ap()
    attn_d = nc.dram_tensor("attn_w", [EMB, EMB], bf16, kind="ExternalInput").ap()
    attnb_d = nc.dram_tensor("attn_b", [P, EMB], f32, kind="ExternalInput").ap()
    ident_d = nc.dram_tensor("ident", [P, P], f32, kind="ExternalInput").ap()
    out_d = nc.dram_tensor("out", [spc, EMB], f32, kind="ExternalOutput").ap()

    with ExitStack() as ctx:
        tc = ctx.enter_context(tile.TileContext(nc))
        const = ctx.enter_context(tc.tile_pool(name="const", bufs=1))
        vpool = ctx.enter_context(tc.tile_pool(name="vpool", bufs=5))
        sepool = ctx.enter_context(tc.tile_pool(name="sepool", bufs=4))
        pepool = ctx.enter_context(tc.tile_pool(name="pepool", bufs=4))
        prodpool = ctx.enter_context(tc.tile_pool(name="prodpool", bufs=3))
        ohpool = ctx.enter_context(tc.tile_pool(name="ohpool", bufs=4))
        opool = ctx.enter_context(tc.tile_pool(name="opool", bufs=2))
        dram = ctx.enter_context(tc.tile_pool(name="dram", bufs=1, space="DRAM"))
        psum_uw = ctx.enter_context(tc.tile_pool(name="psum_uw", bufs=2,
                                                 space="PSUM"))
        psum_t = ctx.enter_context(tc.tile_pool(name="psum_t", bufs=1,
                                                space="PSUM"))
        psum_d = ctx.enter_context(tc.tile_pool(name="psum_d", bufs=1,
                                                space="PSUM"))
        psum_z = ctx.enter_context(tc.tile_pool(name="psum_z", bufs=2,
                                                space="PSUM"))
        psum_s = ctx.enter_context(tc.tile_pool(name="psum_s", bufs=1,
                                                space="PSUM"))
        vthpool = ctx.enter_context(tc.tile_pool(name="vthpool", bufs=4))
        s2pool = ctx.enter_context(tc.tile_pool(name="s2pool", bufs=4))
        stpool = ctx.enter_context(tc.tile_pool(name="stpool", bufs=2))

        # ---- constants ----
        gate_sb = const.tile([P, 1, EMB], bf16, tag="gate_sb")
        nc.sync.dma_start(out=gate_sb[:, 0, :], in_=gate_d)
        attn0_sb = const.tile([P, EMB], bf16, tag="attn0")
        nc.sync.dma_start(out=attn0_sb, in_=attn_d[0:HALF, :])
        attn1_sb = const.tile([P, EMB], bf16, tag="attn1")
        nc.sync.dma_start(out=attn1_sb, in_=attn_d[HALF:EMB, :])
        attnb_sb = const.tile([P, EMB], f32, tag="attnb")
        nc.sync.dma_start(out=attnb_sb, in_=attnb_d)
        ident_sb = const.tile([P, P], f32, tag="ident")
        nc.sync.dma_start(out=ident_sb, in_=ident_d)

        scratch_act = const.tile([P, EMB], bf16, tag="scratch_act")
        gate2_sb = const.tile([PE_W, 1], bf16, tag="gate2_sb")
        nc.sync.dma_start(out=gate2_sb, in_=gate2_d)
        u_stage0 = const.tile([P, n_grp * GRP], bf16, tag="u_stage0")
        u_stage1 = const.tile([P, n_grp * GRP], bf16, tag="u_stage1")
        d_cols = const.tile([SEGW, W], f32, tag="d_cols")
        z_stage = const.tile([P, n_grp, EMB], f32, tag="z_stage")

        def one_pass(rep):
            vt_tiles = [None] * n_dma
            pe_tiles = [None] * n_dma

            def ensure_group(g):
                if vt_tiles[g] is not None:
                    return
                vt = vpool.tile([P, BLK, COLS], bf16, tag="vt")
                nc.sync.dma_start(
                    out=vt.rearrange("p n d -> p (n d)"),
                    in_=v_d[g * P : (g + 1) * P, :],
                )
                oh_g = ohpool.tile([P, BLK, SEGW], bf16, tag="oh_g")
                nc.sync.dma_start(
                    out=oh_g.rearrange("p n s -> p (n s)"),
                    in_=oh_d[:, g * BLK * SEGW : (g + 1) * BLK * SEGW],
                )
                # ---- PE half of the gate-dot: s2 = v[:,128:256] . g2 ----
                ncols = P * BLK
                vth_g = vthpool.tile([PE_W, ncols], bf16, tag="vth_g")
                nc.scalar.dma_start(out=vth_g,
                                    in_=vth_d[:, g * ncols:(g + 1) * ncols])
                s2_ps = psum_s.tile([P, BLK], f32, tag="s2_ps")
                for j in range(BLK):
                    nc.tensor.matmul(s2_ps[:, j : j + 1],
                                     lhsT=vth_g[:, j * P:(j + 1) * P],
                                     rhs=gate2_sb,
                                     start=True, stop=True)
                # ---- gate dot products for the whole group ----
                # v1-style minimal-SBUF-traffic split: DVE mults MULT_DVE
                # blocks + reduces RED_DVE; GPSIMD mults the rest; ACT
                # reduces the rest via Copy+accum.
                prod = prodpool.tile([P, BLK, VEC_W], bf16, tag="prod")
                a_v, a_gate = broadcast_tensor_aps(
                    vt[:, 0:MULT_DVE, 0:VEC_W], gate_sb[:, :, 0:VEC_W])
                nc.vector.tensor_mul(prod[:, 0:MULT_DVE, :], a_v, a_gate)
                if MULT_DVE < BLK:
                    b_v, b_gate = broadcast_tensor_aps(
                        vt[:, MULT_DVE:BLK, 0:VEC_W], gate_sb[:, :, 0:VEC_W])
                    nc.gpsimd.tensor_mul(prod[:, MULT_DVE:BLK, :], b_v, b_gate)
                s_g = sepool.tile([P, BLK], fp16, tag="s_g")
                with nc.allow_low_precision("s reduce"):
                    nc.vector.tensor_reduce(
                        out=s_g[:, 0:RED_DVE], in_=prod[:, 0:RED_DVE, :],
                        axis=mybir.AxisListType.X, op=Alu.add)
                for j in range(RED_DVE, BLK):
                    nc.scalar.activation(
                        scratch_act[:, 0:VEC_W], prod[:, j, :], Act.Copy,
                        accum_out=s_g[:, j : j + 1])
                s_sum = sepool.tile([P, BLK], f32, tag="s_sum")
                nc.vector.tensor_add(s_sum, s_g, s2_ps)
                e_g = sepool.tile([P, BLK], bf16, tag="e_g")
                nc.scalar.activation(e_g, s_sum, Act.Exp)
                # pe = onehot * e (GPSIMD broadcast multiply)
                pe_g = pepool.tile([P, BLK, SEGW], bf16, tag="pe_g")
                e_ap = e_g[:, :]
                e_3d = bass.AP(e_ap.tensor, e_ap.offset,
                               [list(d) for d in e_ap.ap] + [[1, 1]])
                a_oh, a_e = broadcast_tensor_aps(oh_g[:, :, :], e_3d)
                nc.gpsimd.tensor_mul(pe_g[:, :, :], a_oh, a_e)
                vt_tiles[g] = vt
                pe_tiles[g] = pe_g

            def stage_z(fg):
                # attn projection for one 128-seg group as soon as its 8
                # windows are staged; the 1/D scale happens at the end.
                lo = fg * GRP
                z = psum_z.tile([GRP, EMB], f32, tag="z")
                nc.tensor.matmul(z, lhsT=u_stage0[:, lo : lo + GRP],
                                 rhs=attn0_sb, start=True, stop=False)
                nc.tensor.matmul(z, lhsT=u_stage1[:, lo : lo + GRP],
                                 rhs=attn1_sb, start=False, stop=True)
                nc.scalar.copy(z_stage[:, fg, :], z)

            gb = 0
            for w in range(W):
                segw = win_w[w]
                uw = psum_uw.tile([SEGW, EMBA], f32, tag="uw")
                for b in range(b_w[w]):
                    g, j = divmod(gb, BLK)
                    ensure_group(g)
                    nc.tensor.matmul(uw, lhsT=pe_tiles[g][:, j, :],
                                     rhs=vt_tiles[g][:, j, 0:EMBA],
                                     start=(b == 0), stop=(b == b_w[w] - 1))
                    gb += 1
                # ---- window epilogue ----
                off = win_lo[w]
                u_sb = stpool.tile([SEGW, EMBA], f32, tag="u_sb")
                nc.scalar.copy(u_sb, uw)
                t0p = psum_t.tile([P, SEGW], f32, tag="t0p")
                nc.tensor.transpose(t0p, u_sb[:, 0:HALF],
                                    ident_sb[0:SEGW, 0:SEGW])
                t1p = psum_t.tile([P, SEGW], f32, tag="t1p")
                nc.tensor.transpose(t1p, u_sb[:, HALF:EMB],
                                    ident_sb[0:SEGW, 0:SEGW])
                nc.scalar.copy(u_stage0[:, off : off + segw], t0p[:, 0:segw])
                nc.scalar.copy(u_stage1[:, off : off + segw], t1p[:, 0:segw])
                nc.scalar.copy(d_cols[:, w : w + 1], u_sb[:, EMB:EMBA])

            # ---- D: [seg-in-window, window] -> per-partition layout via
            # DRAM roundtrip + PE transpose, then out = z/D + bias. The
            # roundtrip latency hides under the z-staging matmuls. ----
            d_dram = dram.tile([SEGW, W], f32, tag="d_dram")
            nc.sync.dma_start(out=d_dram, in_=d_cols)
            d_rows = opool.tile([P, GRP], f32, tag="d_rows")
            nc.vector.memset(d_rows, 0.0)
            nc.sync.dma_start(
                out=d_rows[0:n_grp, :].rearrange("g (a r) -> g a r", r=SEGW),
                in_=d_dram.rearrange("r (g a) -> g a r", g=n_grp),
            )
            for fg in range(n_grp):
                stage_z(fg)
            dT = psum_d.tile([P, P], f32, tag="dT")
            nc.tensor.transpose(dT, d_rows, ident_sb)
            d_cl = opool.tile([P, n_grp], f32, tag="d_cl")
            nc.vector.tensor_scalar_max(d_cl, dT[:, 0:n_grp], 1e-30)
            rec = opool.tile([P, n_grp], f32, tag="rec")
            nc.vector.reciprocal(rec, d_cl)
            for fg in range(n_grp):
                lo = fg * GRP
                o_sb = opool.tile([GRP, EMB], f32, tag="o_sb")
                nc.scalar.activation(o_sb, z_stage[:, fg, :], Act.Copy,
                                     scale=rec[:, fg : fg + 1])
                nc.vector.tensor_add(o_sb, o_sb, attnb_sb)
                nc.sync.dma_start(out=out_d[lo : lo + GRP, :], in_=o_sb)

        for rep in range(reps):
            one_pass(rep)

    nc.compile()
    return nc


def _get_program(meta, reps=1):
    key = (meta["W"], meta["b_w"], meta["win_lo"], meta["win_w"],
           meta["spc"], reps)
    if key not in _CACHE:
        _CACHE[key] = build_bass(meta, reps=reps)
    return _CACHE[key]


def make_const_inputs(gate_w, attn_w, attn_b):
    gate_rep = np.ascontiguousarray(
        np.broadcast_to(np.asarray(gate_w, np.float32).reshape(1, EMB),
                        (P, EMB))).astype(ml_dtypes.bfloat16)
    return {
        "gate_rep": gate_rep,
        "gate2": np.ascontiguousarray(
            np.asarray(gate_w, np.float32).reshape(EMB, 1)[VEC_W:EMB]
        ).astype(ml_dtypes.bfloat16),
        "attn_w": np.asarray(attn_w, np.float32).astype(ml_dtypes.bfloat16),
        "attn_b": np.ascontiguousarray(np.broadcast_to(
            np.asarray(attn_b, np.float32).reshape(1, EMB), (P, EMB))),
        "ident": np.eye(P, dtype=np.float32),
    }


def build_in_maps(values, indices, num_graphs, gate_w, attn_w, attn_b):
    G = int(num_graphs)
    per_core, meta = prepare_host(values, indices, G)
    consts = make_const_inputs(gate_w, attn_w, attn_b)
    in_maps = [{**consts, "v": pc["v"], "oh": pc["oh"], "vth": pc["vth"]}
               for pc in per_core]
    return in_maps, meta


# ----------------------------------------------------------------------------
# Public entry point.
# ----------------------------------------------------------------------------
def kernel(values, indices, num_graphs, gate_w, gate_b, attn_w, attn_b):
    from concourse.bass_utils import run_bass_kernel_spmd

    in_maps, meta = build_in_maps(values, indices, num_graphs,
                                  gate_w, attn_w, attn_b)
    nc = _get_program(meta)
    res = run_bass_kernel_spmd(nc, in_maps, core_ids=list(range(NCORES)))
    out = np.concatenate([res.results[c]["out"] for c in range(NCORES)], axis=0)
    return out[: int(num_graphs)]


# revision 37
# speedup vs baseline: 1.0655x; 1.0655x over previous
"""Trainium2 Bass kernel for AttentionalAggregation (segment softmax-weighted sum).

reference math:
    s = values @ gate_w + gate_b            # [N,1]
    w = segment_softmax(s, indices)         # [N,1]
    out = segment_sum(w * (values @ attn_w + attn_b))   # [G,EMB]

Algebraic restructuring (exact up to fp rounding):
  softmax weights per segment sum to 1, so
      out[g] = (U[g]/D[g]) @ attn_w + attn_b
  with U[g] = sum_{i in g} e_i * values_i, D[g] = sum_{i in g} e_i,
  e_i = exp(values_i . gate_w).  gate_b and the per-segment max shift
  cancel in the U/D ratio (|s| <= ~4 for this data, exp can't overflow).

Sharding: indices are sorted, so each of the 8 cores owns G/8 contiguous
segments and their (contiguous) nodes. No collectives.

v2 layout: each 128-node block carries 258 bf16 columns
  [ v (256) | 1.0 (1) | one-hot (16) ]
so ONE ~2.2MB DMA per 32-block group feeds everything (v for the PE
matmul rhs incl. the D ones-column, one-hot for the segment masks).

Per group the gate-dot s[p, j] = v . gate_w runs as:
  - DVE batched tensor_mul  prod = v * gate        (bf16 2x mode)
  - fold tree: prod 256 -> 128 -> 64 -> 32 by pairwise adds, with the
    first fold split DVE/GPSIMD by block ranges (tunable balance), the
    rest on the two engines by range
  - one batched DVE tensor_reduce [128, 32, 32] -> s (1x but only 32 wide)
This costs ~2x less DVE time than mult + full-width 1x reduce.

ACT: exp per group; GPSIMD: pe = onehot * e broadcast-multiply.
PE per block: LDWEIGHTS pe[:, j, :] (16 cols) + MATMUL uw[16, 257] over
rhs [v|1] accumulated across the window's blocks in PSUM; window epilogue
transposes U into per-core staging and copies the D column; every 8
windows one 128-segment output group runs (2 matmuls + 1/D scale + bias
+ DMA out), overlapping the tail with the stream.
"""

import numpy as np
import ml_dtypes

P = 128
EMB = 256
EMBA = EMB + 1      # v plus the ones column
COLS = EMBA + 1     # v + ones + pad (4B-aligned block strides); oh ships separately
HALF = 128
SEGW = 16           # segments per window == one-hot width
NCORES = 8
BLK = 16            # blocks per DMA group
GRP = 128           # segments per final-matmul group
MULT_DVE = 11       # blocks whose gate-dot multiply runs on DVE (rest GPSIMD)
RED_DVE = 16        # blocks whose free-dim reduce runs on DVE (rest ACT)
VEC_W = 128         # emb width of the vector gate-dot path; PE covers the rest
PE_W = EMB - VEC_W

_CACHE = {}


# ----------------------------------------------------------------------------
# Host-side preparation: shard + pad nodes into (core, window, block) layout.
# ----------------------------------------------------------------------------
def prepare_host(values, indices, G):
    idx = np.ascontiguousarray(np.asarray(indices).astype(np.int64))
    counts = np.bincount(idx, minlength=G)
    seg_start = np.zeros(G + 1, dtype=np.int64)
    np.cumsum(counts, out=seg_start[1:])

    assert G % NCORES == 0
    spc = G // NCORES                      # segments per core
    win_lo = list(range(0, spc, SEGW))
    win_w = [min(SEGW, spc - lo) for lo in win_lo]
    W = len(win_lo)

    b_w = []
    for w in range(W):
        need = 1
        for c in range(NCORES):
            s0 = c * spc + win_lo[w]
            n = int(seg_start[s0 + win_w[w]] - seg_start[s0])
            need = max(need, (n + P - 1) // P)
        b_w.append(need)
    nblk = sum(b_w)

    vals = np.asarray(values, dtype=np.float32)
    n_dma = (nblk + BLK - 1) // BLK
    nblk_pad = n_dma * BLK
    per_core = []
    for c in range(NCORES):
        v_pad = np.zeros((nblk_pad * P, COLS), dtype=ml_dtypes.bfloat16)
        oh = np.zeros((P, nblk_pad, SEGW), dtype=ml_dtypes.bfloat16)
        gb = 0
        for w in range(W):
            s0 = c * spc + win_lo[w]
            lo = int(seg_start[s0])
            hi = int(seg_start[s0 + win_w[w]])
            r = lo
            for b in range(b_w[w]):
                n = min(P, hi - r)
                if n > 0:
                    rows = slice(gb * P, gb * P + n)
                    v_pad[rows, 0:EMB] = vals[r : r + n]
                    v_pad[rows, EMB] = 1.0
                    loc = (idx[r : r + n] - s0).astype(np.int64)
                    oh[np.arange(n), gb, loc] = 1.0
                r += n
                gb += 1
        # regroup so each DMA group's data is per-partition-contiguous:
        # [g, n, p, d] -> [g, p, n, d]
        v_pad = np.ascontiguousarray(
            v_pad.reshape(n_dma, BLK, P, COLS).transpose(0, 2, 1, 3)
        ).reshape(n_dma * P, BLK * COLS)
        oh = np.ascontiguousarray(oh).reshape(P, nblk_pad * SEGW)
        # v^T for emb[128:256]: partition = emb dim; within group g columns
        # run (j * P + p) so block j's 128 columns form the LDWEIGHTS tile
        # whose gate2 matvec yields s2 for block j partition-parallel.
        v5 = v_pad.reshape(n_dma, P, BLK, COLS)
        vth = np.ascontiguousarray(
            v5[:, :, :, VEC_W:EMB].transpose(3, 0, 2, 1)
        ).reshape(PE_W, n_dma * BLK * P)
        per_core.append({"v": v_pad, "oh": oh, "vth": vth})
    meta = {"W": W, "b_w": tuple(b_w), "win_lo": tuple(win_lo),
            "win_w": tuple(win_w), "nblk": nblk, "spc": spc, "n_dma": n_dma}
    return per_core, meta


# ----------------------------------------------------------------------------
# Bass program (identical for all cores; data differs per core).
# ----------------------------------------------------------------------------
def build_bass(meta, reps=1):
    import concourse.bass as bass
    import concourse.bacc as bacc
    import concourse.tile as tile
    from concourse import mybir
    from concourse.bass import broadcast_tensor_aps
    from contextlib import ExitStack

    f32 = mybir.dt.float32
    bf16 = mybir.dt.bfloat16
    fp16 = mybir.dt.float16
    Act = mybir.ActivationFunctionType
    Alu = mybir.AluOpType

    W = meta["W"]
    b_w = meta["b_w"]
    win_lo = meta["win_lo"]
    win_w = meta["win_w"]
    nblk = meta["nblk"]
    spc = meta["spc"]
    n_dma = meta["n_dma"]
    n_grp = (spc + GRP - 1) // GRP
    win_per_grp = GRP // SEGW
    assert spc % GRP == 0 and W * SEGW == spc

    nc = bacc.Bacc(
        "TRN2",
        target_bir_lowering=False,
        debug=False,
        enable_asserts=False,
        num_devices=NCORES,
    )

    v_d = nc.dram_tensor("v", [n_dma * P, BLK * COLS], bf16,
                         kind="ExternalInput").ap()
    oh_d = nc.dram_tensor("oh", [P, n_dma * BLK * SEGW], bf16,
                          kind="ExternalInput").ap()
    vth_d = nc.dram_tensor("vth", [PE_W, n_dma * P * BLK], bf16,
                           kind="ExternalInput").ap()
    gate2_d = nc.dram_tensor("gate2", [PE_W, 1], bf16, kind="ExternalInput").ap()
    gate_d = nc.dram_tensor("gate_rep", [P, EMB], bf16, kind="ExternalInput").ap()
    attn_d = nc.dram_tensor("attn_w", [EMB, EMB], bf16, kind="ExternalInput").ap()
    attnb_d = nc.dram_tensor("attn_b", [P, EMB], f32, kind="ExternalInput").ap()
    ident_d = nc.dram_tensor("ident", [P, P], f32, kind="ExternalInput").ap()
    out_d = nc.dram_tensor("out", [spc, EMB], f32, kind="ExternalOutput").ap()

    with ExitStack() as ctx:
        tc = ctx.enter_context(tile.TileContext(nc))
        const = ctx.enter_context(tc.tile_pool(name="const", bufs=1))
        vpool = ctx.enter_context(tc.tile_pool(name="vpool", bufs=5))
        sepool = ctx.enter_context(tc.tile_pool(name="sepool", bufs=4))
        pepool = ctx.enter_context(tc.tile_pool(name="pepool", bufs=4))
        prodpool = ctx.enter_context(tc.tile_pool(name="prodpool", bufs=3))
        ohpool = ctx.enter_context(tc.tile_pool(name="ohpool", bufs=4))
        opool = ctx.enter_context(tc.tile_pool(name="opool", bufs=2))
        dram = ctx.enter_context(tc.tile_pool(name="dram", bufs=1, space="DRAM"))
        psum_uw = ctx.enter_context(tc.tile_pool(name="psum_uw", bufs=2,
                                                 space="PSUM"))
        psum_t = ctx.enter_context(tc.tile_pool(name="psum_t", bufs=1,
                                                space="PSUM"))
        psum_d = ctx.enter_context(tc.tile_pool(name="psum_d", bufs=1,
                                                space="PSUM"))
        psum_z = ctx.enter_context(tc.tile_pool(name="psum_z", bufs=2,
                                                space="PSUM"))
        psum_s = ctx.enter_context(tc.tile_pool(name="psum_s", bufs=1,
                                                space="PSUM"))
        vthpool = ctx.enter_context(tc.tile_pool(name="vthpool", bufs=4))
        s2pool = ctx.enter_context(tc.tile_pool(name="s2pool", bufs=4))
        stpool = ctx.enter_context(tc.tile_pool(name="stpool", bufs=2))

        # ---- constants ----
        gate_sb = const.tile([P, 1, EMB], bf16, tag="gate_sb")
        nc.sync.dma_start(out=gate_sb[:, 0, :], in_=gate_d)
        attn0_sb = const.tile([P, EMB], bf16, tag="attn0")
        nc.sync.dma_start(out=attn0_sb, in_=attn_d[0:HALF, :])
        attn1_sb = const.tile([P, EMB], bf16, tag="attn1")
        nc.sync.dma_start(out=attn1_sb, in_=attn_d[HALF:EMB, :])
        attnb_sb = const.tile([P, EMB], f32, tag="attnb")
        nc.sync.dma_start(out=attnb_sb, in_=attnb_d)
        ident_sb = const.tile([P, P], f32, tag="ident")
        nc.sync.dma_start(out=ident_sb, in_=ident_d)

        scratch_act = const.tile([P, EMB], bf16, tag="scratch_act")
        gate2_sb = const.tile([PE_W, 1], bf16, tag="gate2_sb")
        nc.sync.dma_start(out=gate2_sb, in_=gate2_d)
        u_stage0 = const.tile([P, n_grp * GRP], bf16, tag="u_stage0")
        u_stage1 = const.tile([P, n_grp * GRP], bf16, tag="u_stage1")
        d_cols = const.tile([SEGW, W], f32, tag="d_cols")
        z_stage = const.tile([P, n_grp, EMB], f32, tag="z_stage")

        def one_pass(rep):
            vt_tiles = [None] * n_dma
            pe_tiles = [None] * n_dma

            def ensure_group(g):
                if vt_tiles[g] is not None:
                    return
                vt = vpool.tile([P, BLK, COLS], bf16, tag="vt")
                nc.sync.dma_start(
                    out=vt.rearrange("p n d -> p (n d)"),
                    in_=v_d[g * P : (g + 1) * P, :],
                )
                oh_g = ohpool.tile([P, BLK, SEGW], bf16, tag="oh_g")
                nc.sync.dma_start(
                    out=oh_g.rearrange("p n s -> p (n s)"),
                    in_=oh_d[:, g * BLK * SEGW : (g + 1) * BLK * SEGW],
                )
                # ---- PE half of the gate-dot: s2 = v[:,128:256] . g2 ----
                ncols = P * BLK
                vth_g = vthpool.tile([PE_W, ncols], bf16, tag="vth_g")
                nc.scalar.dma_start(out=vth_g,
                                    in_=vth_d[:, g * ncols:(g + 1) * ncols])
                s2_ps = psum_s.tile([P, BLK], f32, tag="s2_ps")
                for j in range(BLK):
                    nc.tensor.matmul(s2_ps[:, j : j + 1],
                                     lhsT=vth_g[:, j * P:(j + 1) * P],
                                     rhs=gate2_sb,
                                     start=True, stop=True)
                # ---- gate dot products for the whole group ----
                # v1-style minimal-SBUF-traffic split: DVE mults MULT_DVE
                # blocks + reduces RED_DVE; GPSIMD mults the rest; ACT
                # reduces the rest via Copy+accum.
                prod = prodpool.tile([P, BLK, VEC_W], bf16, tag="prod")
                a_v, a_gate = broadcast_tensor_aps(
                    vt[:, 0:MULT_DVE, 0:VEC_W], gate_sb[:, :, 0:VEC_W])
                nc.vector.tensor_mul(prod[:, 0:MULT_DVE, :], a_v, a_gate)
                if MULT_DVE < BLK:
                    b_v, b_gate = broadcast_tensor_aps(
                        vt[:, MULT_DVE:BLK, 0:VEC_W], gate_sb[:, :, 0:VEC_W])
                    nc.gpsimd.tensor_mul(prod[:, MULT_DVE:BLK, :], b_v, b_gate)
                s_g = sepool.tile([P, BLK], fp16, tag="s_g")
                with nc.allow_low_precision("s reduce"):
                    nc.vector.tensor_reduce(
                        out=s_g[:, 0:RED_DVE], in_=prod[:, 0:RED_DVE, :],
                        axis=mybir.AxisListType.X, op=Alu.add)
                for j in range(RED_DVE, BLK):
                    nc.scalar.activation(
                        scratch_act[:, 0:VEC_W], prod[:, j, :], Act.Copy,
                        accum_out=s_g[:, j : j + 1])
                s_sum = sepool.tile([P, BLK], f32, tag="s_sum")
                nc.vector.tensor_add(s_sum, s_g, s2_ps)
                e_g = sepool.tile([P, BLK], bf16, tag="e_g")
                nc.scalar.activation(e_g, s_sum, Act.Exp)
                # pe = onehot * e (GPSIMD broadcast multiply)
                pe_g = pepool.tile([P, BLK, SEGW], bf16, tag="pe_g")
                e_ap = e_g[:, :]
                e_3d = bass.AP(e_ap.tensor, e_ap.offset,
                               [list(d) for d in e_ap.ap] + [[1, 1]])
                a_oh, a_e = broadcast_tensor_aps(oh_g[:, :, :], e_3d)
                nc.gpsimd.tensor_mul(pe_g[:, :, :], a_oh, a_e)
                vt_tiles[g] = vt
                pe_tiles[g] = pe_g

            def stage_z(fg):
                # attn projection for one 128-seg group as soon as its 8
                # windows are staged; the 1/D scale happens at the end.
                lo = fg * GRP
                z = psum_z.tile([GRP, EMB], f32, tag="z")
                nc.tensor.matmul(z, lhsT=u_stage0[:, lo : lo + GRP],
                                 rhs=attn0_sb, start=True, stop=False)
                nc.tensor.matmul(z, lhsT=u_stage1[:, lo : lo + GRP],
                                 rhs=attn1_sb, start=False, stop=True)
                nc.scalar.copy(z_stage[:, fg, :], z)

            gb = 0
            for w in range(W):
                segw = win_w[w]
                uw = psum_uw.tile([SEGW, EMBA], f32, tag="uw")
                for b in range(b_w[w]):
                    g, j = divmod(gb, BLK)
                    ensure_group(g)
                    nc.tensor.matmul(uw, lhsT=pe_tiles[g][:, j, :],
                                     rhs=vt_tiles[g][:, j, 0:EMBA],
                                     start=(b == 0), stop=(b == b_w[w] - 1))
                    gb += 1
                # ---- window epilogue ----
                off = win_lo[w]
                u_sb = stpool.tile([SEGW, EMBA], f32, tag="u_sb")
                nc.scalar.copy(u_sb, uw)
                t0p = psum_t.tile([P, SEGW], f32, tag="t0p")
                nc.tensor.transpose(t0p, u_sb[:, 0:HALF],
                                    ident_sb[0:SEGW, 0:SEGW])
                t1p = psum_t.tile([P, SEGW], f32, tag="t1p")
                nc.tensor.transpose(t1p, u_sb[:, HALF:EMB],
                                    ident_sb[0:SEGW, 0:SEGW])
                nc.scalar.copy(u_stage0[:, off : off + segw], t0p[:, 0:segw])
                nc.scalar.copy(u_stage1[:, off : off + segw], t1p[:, 0:segw])
                nc.scalar.copy(d_cols[:, w : w + 1], u_sb[:, EMB:EMBA])

            # ---- D: [seg-in-window, window] -> per-partition layout via
            # DRAM roundtrip + PE transpose, then out = z/D + bias. The
            # roundtrip latency hides under the z-staging matmuls. ----
            d_dram = dram.tile([SEGW, W], f32, tag="d_dram")
            nc.sync.dma_start(out=d_dram, in_=d_cols)
            d_rows = opool.tile([P, GRP], f32, tag="d_rows")
            nc.vector.memset(d_rows, 0.0)
            nc.sync.dma_start(
                out=d_rows[0:n_grp, :].rearrange("g (a r) -> g a r", r=SEGW),
                in_=d_dram.rearrange("r (g a) -> g a r", g=n_grp),
            )
            for fg in range(n_grp):
                stage_z(fg)
            dT = psum_d.tile([P, P], f32, tag="dT")
            nc.tensor.transpose(dT, d_rows, ident_sb)
            d_cl = opool.tile([P, n_grp], f32, tag="d_cl")
            nc.vector.tensor_scalar_max(d_cl, dT[:, 0:n_grp], 1e-30)
            rec = opool.tile([P, n_grp], f32, tag="rec")
            nc.vector.reciprocal(rec, d_cl)
            for fg in range(n_grp):
                lo = fg * GRP
                o_sb = opool.tile([GRP, EMB], f32, tag="o_sb")
                nc.scalar.activation(o_sb, z_stage[:, fg, :], Act.Copy,
                                     scale=rec[:, fg : fg + 1])
                nc.vector.tensor_add(o_sb, o_sb, attnb_sb)
                nc.sync.dma_start(out=out_d[lo : lo + GRP, :], in_=o_sb)

        for rep in range(reps):
            one_pass(rep)

    nc.compile()
    return nc


def _get_program(meta, reps=1):
    key = (meta["W"], meta["b_w"], meta["win_lo"], meta["win_w"],
           meta["spc"], reps)
    if key not in _CACHE:
        _CACHE[key] = build_bass(meta, reps=reps)
    return _CACHE[key]


def make_const_inputs(gate_w, attn_w, attn_b):
    gate_rep = np.ascontiguousarray(
        np.broadcast_to(np.asarray(gate_w, np.float32).reshape(1, EMB),
                        (P, EMB))).astype(ml_dtypes.bfloat16)
    return {
        "gate_rep": gate_rep,
        "gate2": np.ascontiguousarray(
            np.asarray(gate_w, np.float32).reshape(EMB, 1)[VEC_W:EMB]
        ).astype(ml_dtypes.bfloat16),
        "attn_w": np.asarray(attn_w, np.float32).astype(ml_dtypes.bfloat16),
        "attn_b": np.ascontiguousarray(np.broadcast_to(
            np.asarray(attn_b, np.float32).reshape(1, EMB), (P, EMB))),
        "ident": np.eye(P, dtype=np.float32),
    }


def build_in_maps(values, indices, num_graphs, gate_w, attn_w, attn_b):
    G = int(num_graphs)
    per_core, meta = prepare_host(values, indices, G)
    consts = make_const_inputs(gate_w, attn_w, attn_b)
    in_maps = [{**consts, "v": pc["v"], "oh": pc["oh"], "vth": pc["vth"]}
               for pc in per_core]
    return in_maps, meta


# ----------------------------------------------------------------------------
# Public entry point.
# ----------------------------------------------------------------------------
def kernel(values, indices, num_graphs, gate_w, gate_b, attn_w, attn_b):
    from concourse.bass_utils import run_bass_kernel_spmd

    in_maps, meta = build_in_maps(values, indices, num_graphs,
                                  gate_w, attn_w, attn_b)
    nc = _get_program(meta)
    res = run_bass_kernel_spmd(nc, in_maps, core_ids=list(range(NCORES)))
    out = np.concatenate([res.results[c]["out"] for c in range(NCORES)], axis=0)
    return out[: int(num_graphs)]
